# revision 2
# baseline (speedup 1.0000x reference)
"""GAT (graph attention) full-graph kernel for 8 Trainium2 NeuronCores.

Strategy (dst-sharded graph parallel, host as interconnect):
  Launch 1 (SPMD, node-sharded): core k projects its 12,500 nodes in
    512-column chunks: featT[hc, n] = W^T x^T (bf16 matmul, fp32 psum),
    el/er[l, n] = alr^T featT (fp16 out).
  Host (pure integer routing + byte gathers of device-computed arrays):
    assigns each core's nodes to 98 dst tiles (load balancing), routes every
    edge to the core owning its dst, pads to common 128-edge blocks per dst
    tile, and lays out per-edge feat[src] / el[src] / er[dst] rows in block
    order (np.take of launch-1 output bytes; no float arithmetic).
  Launch 2 (SPMD, dst-sharded): streams the pre-permuted edge blocks
    sequentially (line-rate DMA, no device-side gather):
    ex = exp(leakyrelu(el+er)) (DVE+ACT),
    mt[e, d] = (dstoff[e]==d) one-hot (DVE compare vs iota),
    psum[d, 0:132] += mt^T @ [feat*ex | ex] (PE, accumulated per dst tile),
    epilogue: out[d] = relu(mean_h(numer_h / s_h) + mean(bias)).

Self-contained: hardcodes problem shapes; all float arithmetic on-device.
"""

import numpy as np
import ml_dtypes

import concourse.bacc as bacc
import concourse.bass as bass
import concourse.mybir as mybir
import concourse.tile as tile
from concourse.bass_utils import run_bass_kernel_spmd
from concourse.bass_interp import get_hw_module

F32 = mybir.dt.float32
BF16 = mybir.dt.bfloat16
F16 = mybir.dt.float16

# ---- problem constants ----
N = 100000
H = 4
C = 32
E = 1600000
IN = 256
HC = H * C            # 128
NEG = 0.2

NCORES = 8
NPC = N // NCORES     # 12500 nodes per core
CHUNK = 512
NCH = 25              # ceil(12500/512)
NPAD1 = NCH * CHUNK   # 12800 (launch-1 node padding)
TILES = 98            # ceil(12500/128) dst tiles per core
NPAD2 = TILES * 128   # 12544 (launch-2 output padding)
STILE = 4             # dst tiles per supertile (DMA/DVE batching)
NSUP = (TILES + STILE - 1) // STILE  # 25

_cache = {}


# --------------------------------------------------------------------------
# Launch 1: sharded projection (featT + el/er)
# --------------------------------------------------------------------------
def build_launch1():
    nc = bacc.Bacc("TRN2", target_bir_lowering=False, debug=False,
                   num_devices=NCORES)
    xt_d = nc.dram_tensor("xt", [IN, NPAD1], F32, kind="ExternalInput")
    w_d = nc.dram_tensor("w", [IN, HC], F32, kind="ExternalInput")
    alr_d = nc.dram_tensor("alr", [HC, 2 * H], F32, kind="ExternalInput")
    featT_d = nc.dram_tensor("featT", [HC, NPAD1], BF16, kind="ExternalOutput")
    elrT_d = nc.dram_tensor("elrT", [2 * H, NPAD1], F16, kind="ExternalOutput")

    with tile.TileContext(nc) as tc:
        with (
            tc.tile_pool(name="const", bufs=1) as cp,
            tc.tile_pool(name="work", bufs=3) as wp,
            tc.tile_pool(name="psf", bufs=2, space="PSUM") as psf,
            tc.tile_pool(name="pse", bufs=2, space="PSUM") as pse,
        ):
            # W slabs (f32 -> bf16)
            wb = []
            for i in range(2):
                wf = cp.tile([128, HC], F32, tag=f"wf{i}")
                nc.sync.dma_start(wf[:], w_d[i * 128:(i + 1) * 128, :])
                wbi = cp.tile([128, HC], BF16, tag=f"wb{i}")
                nc.scalar.copy(wbi[:], wf[:])
                wb.append(wbi)
            alr_f = cp.tile([HC, 2 * H], F32)
            nc.sync.dma_start(alr_f[:], alr_d[:])
            alr_b = cp.tile([HC, 2 * H], BF16)
            nc.scalar.copy(alr_b[:], alr_f[:])

            for t in range(NCH):
                cs = slice(t * CHUNK, (t + 1) * CHUNK)
                x0 = wp.tile([128, CHUNK], F32, tag="x0")
                nc.sync.dma_start(x0[:], xt_d[0:128, cs])
                x1 = wp.tile([128, CHUNK], F32, tag="x1")
                nc.sync.dma_start(x1[:], xt_d[128:256, cs])
                xb0 = wp.tile([128, CHUNK], BF16, tag="xb0")
                nc.scalar.copy(xb0[:], x0[:])
                xb1 = wp.tile([128, CHUNK], BF16, tag="xb1")
                nc.scalar.copy(xb1[:], x1[:])

                # featT[hc, n] = sum_in W[in, hc] * x^T[in, n]
                pf = psf.tile([128, CHUNK], F32, tag="pf")
                nc.tensor.matmul(pf[:], wb[0][:], xb0[:], start=True, stop=False)
                nc.tensor.matmul(pf[:], wb[1][:], xb1[:], start=False, stop=True)
                fT = wp.tile([128, CHUNK], BF16, tag="fT")
                nc.vector.tensor_copy(fT[:], pf[:])

                # elr[l, n] = sum_hc alr[hc, l] * featT[hc, n]
                pe = pse.tile([2 * H, CHUNK], F32, tag="pe")
                nc.tensor.matmul(pe[:], alr_b[:], fT[:], start=True, stop=True)
                eT = wp.tile([2 * H, CHUNK], F16, tag="eT")
                nc.vector.tensor_copy(eT[:], pe[:])

                nc.sync.dma_start(featT_d[:, cs], fT[:])
                nc.sync.dma_start(elrT_d[:, cs], eT[:])
    nc.compile()
    nc.m = get_hw_module(nc.m)
    return nc


# --------------------------------------------------------------------------
# Launch 2: edge aggregation.  meta = dict with block structure.
# --------------------------------------------------------------------------
def build_launch2(meta):
    nb = meta["nb"]                # [TILES] blocks per tile (common per core)
    NBS_MAX = meta["nbs_max"]      # max blocks in one supertile
    BTOT = meta["btot"]            # total blocks
    sup_base = meta["sup_base"]    # block col base per supertile

    nc = bacc.Bacc("TRN2", target_bir_lowering=False, debug=False,
                   num_devices=NCORES)
    rhs_d = nc.dram_tensor("rhs", [128, BTOT * HC], BF16, kind="ExternalInput")
    elr_d = nc.dram_tensor("elr", [128, BTOT * 2 * H], F16, kind="ExternalInput")
    doff_d = nc.dram_tensor("dstoff", [128, BTOT], BF16, kind="ExternalInput")
    iota_d = nc.dram_tensor("iota", [128, NBS_MAX * 128], BF16,
                            kind="ExternalInput")
    bias_d = nc.dram_tensor("bias", [1, HC], F32, kind="ExternalInput")
    ones_d = nc.dram_tensor("ones", [1, 128], F32, kind="ExternalInput")
    out_d = nc.dram_tensor("out", [NPAD2, C], F32, kind="ExternalOutput")

    with tile.TileContext(nc) as tc:
        with (
            tc.tile_pool(name="const", bufs=1) as cp,
            tc.tile_pool(name="sup", bufs=2) as sp,
            tc.tile_pool(name="bk", bufs=2) as bp,
            tc.tile_pool(name="ep", bufs=2) as ep,
            tc.tile_pool(name="pso", bufs=2 * STILE, space="PSUM") as pso,
        ):
            iota = cp.tile([128, NBS_MAX * 128], BF16)
            nc.sync.dma_start(iota[:], iota_d[:])

            # bias: mean over heads, broadcast to 128 partitions
            bsb = cp.tile([1, HC], F32)
            nc.sync.dma_start(bsb[:], bias_d[:])
            ones = cp.tile([1, 128], F32)
            nc.sync.dma_start(ones[:], ones_d[:])
            b01 = cp.tile([1, C], F32)
            nc.vector.tensor_add(b01[:], bsb[:, 0:C], bsb[:, C:2 * C])
            b23 = cp.tile([1, C], F32)
            nc.vector.tensor_add(b23[:], bsb[:, 2 * C:3 * C], bsb[:, 3 * C:4 * C])
            bsum = cp.tile([1, C], F32)
            nc.vector.tensor_add(bsum[:], b01[:], b23[:])
            bmean = cp.tile([1, C], F32)
            nc.vector.tensor_scalar_mul(bmean[:], bsum[:], 0.25)
            pb = pso.tile([128, HC + H], F32, tag="pout")
            nc.tensor.matmul(pb[:, 0:C], ones[:], bmean[:], start=True, stop=True)
            biasb = cp.tile([128, C], F32)
            nc.vector.tensor_copy(biasb[:], pb[:, 0:C])

            for s in range(NSUP):
                ts = list(range(s * STILE, min((s + 1) * STILE, TILES)))
                nbs = sum(nb[t] for t in ts)
                if nbs == 0:
                    continue
                cb = sup_base[s]          # global block col base

                rhs_sb = sp.tile([128, NBS_MAX, HC], BF16, tag="rhs")
                nc.sync.dma_start(
                    rhs_sb[:, 0:nbs, :].rearrange("p b c -> p (b c)"),
                    rhs_d[:, cb * HC:(cb + nbs) * HC])
                elr_sb = sp.tile([128, NBS_MAX, 2 * H], F16, tag="elr")
                nc.sync.dma_start(
                    elr_sb[:, 0:nbs, :].rearrange("p b h -> p (b h)"),
                    elr_d[:, cb * 2 * H:(cb + nbs) * 2 * H])
                doff_sb = sp.tile([128, NBS_MAX], BF16, tag="doff")
                nc.sync.dma_start(doff_sb[:, 0:nbs], doff_d[:, cb:cb + nbs])

                # e2 = leakyrelu(el + er)  [128, nbs, H] f32
                e_sb = bp.tile([128, NBS_MAX * H], F32, tag="e")
                nc.vector.tensor_tensor(
                    out=e_sb[:, 0:nbs * H].rearrange("p (b h) -> p b h", h=H),
                    in0=elr_sb[:, 0:nbs, 0:H],
                    in1=elr_sb[:, 0:nbs, H:2 * H],
                    op=mybir.AluOpType.add)
                t1 = bp.tile([128, NBS_MAX * H], F32, tag="t1")
                nc.vector.tensor_scalar_mul(
                    t1[:, 0:nbs * H], e_sb[:, 0:nbs * H], NEG)
                e2 = bp.tile([128, NBS_MAX * H], F32, tag="e2")
                nc.vector.tensor_tensor(
                    out=e2[:, 0:nbs * H], in0=e_sb[:, 0:nbs * H],
                    in1=t1[:, 0:nbs * H], op=mybir.AluOpType.max)
                # ex = exp(e2)  [128, nbs, H] bf16  (ACT)
                ex = bp.tile([128, NBS_MAX * H], BF16, tag="ex")
                nc.scalar.activation(ex[:, 0:nbs * H], e2[:, 0:nbs * H],
                                     mybir.ActivationFunctionType.Exp)

                # one-hot mt [128e, nbs, 128d]
                mt = bp.tile([128, NBS_MAX, 128], BF16, tag="mt")
                doff_bc = bass.AP(
                    doff_sb.tensor, doff_sb[:].offset,
                    [doff_sb[:].ap[0], [1, nbs], [0, 128]])
                nc.vector.tensor_tensor(
                    out=mt[:, 0:nbs, :], in0=doff_bc,
                    in1=iota[:, 0:nbs * 128].rearrange("p (b d) -> p b d", d=128),
                    op=mybir.AluOpType.is_equal)

                # rhs2 = [feat * ex | ex]  (all-bf16 packed -> 2x matmul mode)
                rhs2 = bp.tile([128, NBS_MAX, HC + H], BF16, tag="rhs2")
                ex_bc = bass.AP(
                    ex.tensor, ex[:].offset,
                    [ex[:].ap[0], [H, nbs], [1, H], [0, C]])
                nc.vector.tensor_tensor(
                    out=rhs2[:, 0:nbs, 0:HC].rearrange(
                        "p b (h c) -> p b h c", c=C),
                    in0=rhs_sb[:, 0:nbs, :].rearrange(
                        "p b (h c) -> p b h c", c=C),
                    in1=ex_bc, op=mybir.AluOpType.mult)
                nc.vector.tensor_copy(
                    rhs2[:, 0:nbs, HC:HC + H],
                    ex[:, 0:nbs * H].rearrange("p (b h) -> p b h", h=H))

                # aggregate into per-tile psums
                off = 0
                for t in ts:
                    cnt = nb[t]
                    if cnt == 0:
                        continue
                    pout = pso.tile([128, HC + H], F32, tag="pout",
                                    name=f"pout{t}")
                    for j in range(cnt):
                        jb = off + j
                        nc.tensor.matmul(
                            pout[:], mt[:, jb, :], rhs2[:, jb, :],
                            start=(j == 0), stop=(j == cnt - 1),
                            skip_group_check=True)
                    off += cnt

                    # ---- epilogue ----
                    s4 = ep.tile([128, H], F32, tag="s4")
                    nc.vector.tensor_scalar(
                        out=s4[:], in0=pout[:, HC:HC + H], scalar1=4.0,
                        scalar2=1e-20, op0=mybir.AluOpType.mult,
                        op1=mybir.AluOpType.add)
                    srec = ep.tile([128, H], F32, tag="srec")
                    nc.vector.reciprocal_approx_fast(srec[:], s4[:])
                    scaled = ep.tile([128, H, C], F32, tag="scaled")
                    srec_bc = bass.AP(srec.tensor, srec[:].offset,
                                      [srec[:].ap[0], [1, H], [0, C]])
                    nc.vector.tensor_tensor(
                        out=scaled[:],
                        in0=pout[:, 0:HC].rearrange("p (h c) -> p h c", c=C),
                        in1=srec_bc, op=mybir.AluOpType.mult)
                    h01 = ep.tile([128, C], F32, tag="h01")
                    nc.vector.tensor_add(h01[:], scaled[:, 0, :], scaled[:, 1, :])
                    h23 = ep.tile([128, C], F32, tag="h23")
                    nc.vector.tensor_add(h23[:], scaled[:, 2, :], scaled[:, 3, :])
                    hs = ep.tile([128, C], F32, tag="hs")
                    nc.vector.tensor_add(hs[:], h01[:], h23[:])
                    hb = ep.tile([128, C], F32, tag="hb")
                    nc.vector.tensor_add(hb[:], hs[:], biasb[:])
                    outt = ep.tile([128, C], F32, tag="outt")
                    nc.vector.tensor_scalar_max(outt[:], hb[:], 0.0)
                    nc.sync.dma_start(out_d[t * 128:(t + 1) * 128, :], outt[:])
    nc.compile()
    nc.m = get_hw_module(nc.m)
    return nc


# --------------------------------------------------------------------------
# Host-side routing (pure integer work)
# --------------------------------------------------------------------------
def balance_tiles(owner, dloc):
    """Assign each core's 12,500 nodes to 98 dst tiles (<=128 nodes each),
    balancing per-tile in-edge counts.  Returns perm[NCORES, NPC]:
    local node -> tile*128 + slot."""
    perm = np.zeros((NCORES, NPC), np.int64)
    for k in range(NCORES):
        m = owner == k
        deg = np.bincount(dloc[m], minlength=NPC)
        order = np.argsort(-deg, kind="stable")
        load = np.zeros(TILES, np.int64)
        cnt = np.zeros(TILES, np.int64)
        cap = np.full(TILES, 128, np.int64)
        cap[TILES - 1] = NPC - (TILES - 1) * 128
        assign = np.zeros(NPC, np.int64)
        # LPT greedy: heaviest nodes to least-loaded tile with space
        for n in order:
            avail = np.where(cnt < cap)[0]
            t = avail[np.argmin(load[avail])]
            load[t] += deg[n]
            assign[n] = t
            cnt[t] += 1
        slots = np.zeros(TILES, np.int64)
        for n in range(NPC):
            t = assign[n]
            perm[k, n] = t * 128 + slots[t]
            slots[t] += 1
    return perm


def route_edges(src, dst):
    """Bucket edges by (owner core, dst tile); pad to common 128-edge
    blocks.  Returns meta + per-core slot arrays."""
    src = src.astype(np.int64)
    dst = dst.astype(np.int64)
    owner = dst // NPC
    dloc = dst - owner * NPC

    perm = balance_tiles(owner, dloc)
    slot = perm[owner, dloc]            # balanced slot of each edge's dst
    t_id = slot >> 7
    doff = (slot & 127).astype(np.float32)

    cnt = np.bincount(owner * TILES + t_id,
                      minlength=NCORES * TILES).reshape(NCORES, TILES)
    nb = np.ceil(cnt.max(axis=0) / 128.0).astype(np.int64)    # [TILES]
    btot = int(nb.sum())
    epad = btot * 128

    boff = np.zeros(TILES, np.int64)
    boff[1:] = np.cumsum(nb)[:-1]
    sup_base = []
    nbs_max = 0
    for s in range(NSUP):
        ts = list(range(s * STILE, min((s + 1) * STILE, TILES)))
        sup_base.append(int(boff[ts[0]]))
        nbs_max = max(nbs_max, int(sum(nb[t] for t in ts)))

    key = owner * TILES + t_id
    order = np.argsort(key, kind="stable")
    key_s = key[order]
    doff_s = doff[order]
    src_s = src[order]
    dst_s = dst[order]

    srow_all = np.full((NCORES, epad), N, np.int64)   # N -> zero pad row
    drow_all = np.full((NCORES, epad), N, np.int64)
    dst_all = np.full((NCORES, epad), -1.0, np.float32)

    core_starts = np.searchsorted(key_s // TILES, np.arange(NCORES + 1))
    for k in range(NCORES):
        a, b = core_starts[k], core_starts[k + 1]
        kk = key_s[a:b] % TILES                       # tile ids, sorted
        starts = np.searchsorted(kk, np.arange(TILES))
        rank = np.arange(b - a) - starts[kk]
        pos = boff[kk] * 128 + rank
        srow_all[k, pos] = src_s[a:b]
        drow_all[k, pos] = dst_s[a:b]
        dst_all[k, pos] = doff_s[a:b]

    meta = {
        "nb": nb.tolist(),
        "nbs_max": int(nbs_max),
        "btot": btot,
        "sup_base": sup_base,
    }
    return meta, srow_all, drow_all, dst_all, perm


# --------------------------------------------------------------------------
def kernel(x, src, dst, W, attn_l, attn_r, bias):
    x = np.asarray(x, dtype=np.float32)
    src = np.asarray(src)
    dst = np.asarray(dst)
    W = np.asarray(W, dtype=np.float32)
    attn_l = np.asarray(attn_l, dtype=np.float32)
    attn_r = np.asarray(attn_r, dtype=np.float32)
    bias = np.asarray(bias, dtype=np.float32)

    meta, srow_all, drow_all, dst_all, perm = route_edges(src, dst)

    # ---- launch 1 ----
    if "l1" not in _cache:
        _cache["l1"] = build_launch1()
    nc1 = _cache["l1"]

    xt = np.ascontiguousarray(x.T)                     # [256, 100000]
    alr = np.zeros((HC, 2 * H), np.float32)            # block-diag attn layout
    for h in range(H):
        alr[h * C:(h + 1) * C, h] = attn_l[h]
        alr[h * C:(h + 1) * C, H + h] = attn_r[h]

    in1 = []
    for k in range(NCORES):
        xtk = np.zeros((IN, NPAD1), np.float32)
        xtk[:, :NPC] = xt[:, k * NPC:(k + 1) * NPC]
        in1.append({"xt": xtk, "w": W, "alr": alr})
    res1 = run_bass_kernel_spmd(nc1, in1, list(range(NCORES)))

    # feat rows [N+1, HC] bf16 (+ zero pad row); el/er rows [N+1, H] fp16
    feat_rows = np.zeros((N + 1, HC), ml_dtypes.bfloat16)
    el_rows = np.zeros((N + 1, H), np.float16)
    er_rows = np.zeros((N + 1, H), np.float16)
    for k in range(NCORES):
        cs = slice(k * NPC, (k + 1) * NPC)
        feat_rows[cs] = res1.results[k]["featT"][:, :NPC].T
        elr = res1.results[k]["elrT"][:, :NPC]
        el_rows[cs] = elr[0:H].T
        er_rows[cs] = elr[H:2 * H].T

    # ---- launch 2 inputs (host: pure indexing / byte gathers) ----
    key2 = (meta["btot"], meta["nbs_max"], tuple(meta["nb"]))
    if ("l2", key2) not in _cache:
        _cache[("l2", key2)] = build_launch2(meta)
    nc2 = _cache[("l2", key2)]

    btot = meta["btot"]
    iota = np.tile(np.arange(128, dtype=np.float32),
                   meta["nbs_max"]).reshape(1, -1).repeat(128, 0)
    iota = np.ascontiguousarray(iota.astype(ml_dtypes.bfloat16))
    bias2 = bias.reshape(1, HC)
    ones = np.ones((1, 128), np.float32)

    in2 = []
    for k in range(NCORES):
        srow_T = np.ascontiguousarray(
            srow_all[k].reshape(btot, 128).T)          # [128, btot]
        drow_T = np.ascontiguousarray(drow_all[k].reshape(btot, 128).T)
        rhs = feat_rows.take(srow_T, axis=0)           # [128, btot, 128] bf16
        elr = np.concatenate(
            [el_rows.take(srow_T, axis=0),
             er_rows.take(drow_T, axis=0)], axis=2)    # [128, btot, 8] fp16
        doff = np.ascontiguousarray(
            dst_all[k].reshape(btot, 128).T.astype(ml_dtypes.bfloat16))
        in2.append({
            "rhs": np.ascontiguousarray(rhs.reshape(128, btot * HC)),
            "elr": np.ascontiguousarray(elr.reshape(128, btot * 2 * H)),
            "dstoff": doff,
            "iota": iota,
            "bias": bias2,
            "ones": ones,
        })
    res2 = run_bass_kernel_spmd(nc2, in2, list(range(NCORES)))

    out = np.concatenate(
        [res2.results[k]["out"][perm[k]] for k in range(NCORES)])
    return out.astype(np.float32)


# revision 6
# speedup vs baseline: 3.0733x; 3.0733x over previous
"""GAT (graph attention) full-graph kernel for 8 Trainium2 NeuronCores.

Strategy (dst-sharded graph parallel, host as interconnect):
  Launch 1 (SPMD, node-sharded): core k projects its 12,500 nodes in
    512-column chunks: featT[hc, n] = W^T x^T (bf16 matmul, fp32 psum,
    fp16 out), el/er[l, n] = alr^T featT (fp16 out).  W columns are
    pre-permuted c-major (h fastest) so downstream broadcasts are
    unit-stride.
  Host (pure integer routing + byte gathers of device-computed arrays):
    assigns each core's nodes to 196 64-wide dst tiles (load balancing),
    routes every edge to the core owning its dst, pads to common 128-edge
    blocks per tile, lays out per-edge feat[src] / el[src] / er[dst] rows
    in block order (np.take of launch-1 output bytes), and materializes the
    per-block one-hot dst matrices as fp8 {0,1} (pure structure).
  Launch 2 (SPMD, dst-sharded): streams the pre-permuted edge blocks
    sequentially (line-rate DMA, no device-side gather):
    ex = exp(leakyrelu(el+er)) (DVE+ACT),
    psum[64d, 0:132] += mt8^T @ [feat*ex | ex] (fp8 x fp16 PE matmuls,
    accumulated per dst tile), epilogue batched across supertiles:
    out[d] = relu(mean_h(numer_h / s_h) + mean(bias)).

Self-contained: hardcodes problem shapes; all float arithmetic on-device.
"""

import numpy as np
import ml_dtypes

import concourse.bacc as bacc
import concourse.bass as bass
import concourse.mybir as mybir
import concourse.tile as tile
from concourse.bass_utils import run_bass_kernel_spmd
from concourse.bass_interp import get_hw_module

F32 = mybir.dt.float32
BF16 = mybir.dt.bfloat16
F16 = mybir.dt.float16
FP8 = mybir.dt.float8e4

# ---- problem constants ----
N = 100000
H = 4
C = 32
E = 1600000
IN = 256
HC = H * C            # 128
NEG = 0.2

NCORES = 8
NPC = N // NCORES     # 12500 nodes per core
CHUNK = 512
NCH = 25              # ceil(12500/512)
NPAD1 = NCH * CHUNK   # 12800 (launch-1 node padding)
TW = 64               # dst tile width
TILES = 196           # ceil(12500/64)
NPAD2 = TILES * TW    # 12544
STILE = 3             # dst tiles per supertile
NSUP = (TILES + STILE - 1) // STILE  # 66
SBATCH = 11           # supertiles per epilogue batch (66 = 6*11)

_cache = {}


# --------------------------------------------------------------------------
# Launch 1: sharded projection (featT + el/er)
# --------------------------------------------------------------------------
def build_launch1():
    nc = bacc.Bacc("TRN2", target_bir_lowering=False, debug=False,
                   num_devices=NCORES)
    xt_d = nc.dram_tensor("xt", [IN, NPAD1], F32, kind="ExternalInput")
    w_d = nc.dram_tensor("w", [IN, HC], F32, kind="ExternalInput")
    alr_d = nc.dram_tensor("alr", [HC, 2 * H], F32, kind="ExternalInput")
    featT_d = nc.dram_tensor("featT", [HC, NPAD1], F16, kind="ExternalOutput")
    elrT_d = nc.dram_tensor("elrT", [2 * H, NPAD1], F16, kind="ExternalOutput")

    with tile.TileContext(nc) as tc:
        with (
            tc.tile_pool(name="const", bufs=1) as cp,
            tc.tile_pool(name="work", bufs=4) as wp,
            tc.tile_pool(name="psf", bufs=3, space="PSUM") as psf,
            tc.tile_pool(name="pse", bufs=3, space="PSUM") as pse,
        ):
            # W slabs (f32 -> bf16)
            wb = []
            for i in range(2):
                wf = cp.tile([128, HC], F32, tag=f"wf{i}")
                nc.sync.dma_start(wf[:], w_d[i * 128:(i + 1) * 128, :])
                wbi = cp.tile([128, HC], BF16, tag=f"wb{i}")
                nc.scalar.copy(wbi[:], wf[:])
                wb.append(wbi)
            alr_f = cp.tile([HC, 2 * H], F32)
            nc.sync.dma_start(alr_f[:], alr_d[:])
            alr_b = cp.tile([HC, 2 * H], BF16)
            nc.scalar.copy(alr_b[:], alr_f[:])

            for t in range(NCH):
                cs = slice(t * CHUNK, (t + 1) * CHUNK)
                x0 = wp.tile([128, CHUNK], F32, tag="x0")
                nc.sync.dma_start(x0[:], xt_d[0:128, cs])
                x1 = wp.tile([128, CHUNK], F32, tag="x1")
                nc.sync.dma_start(x1[:], xt_d[128:256, cs])
                xb0 = wp.tile([128, CHUNK], BF16, tag="xb0")
                nc.scalar.copy(xb0[:], x0[:])
                xb1 = wp.tile([128, CHUNK], BF16, tag="xb1")
                nc.vector.tensor_copy(xb1[:], x1[:])

                # featT[hc, n] = sum_in W[in, hc] * x^T[in, n]
                pf = psf.tile([128, CHUNK], F32, tag="pf")
                nc.tensor.matmul(pf[:], wb[0][:], xb0[:], start=True, stop=False)
                nc.tensor.matmul(pf[:], wb[1][:], xb1[:], start=False, stop=True)
                fT = wp.tile([128, CHUNK], F16, tag="fT")
                nc.vector.tensor_copy(fT[:], pf[:])
                fTb = wp.tile([128, CHUNK], BF16, tag="fTb")
                nc.scalar.copy(fTb[:], pf[:])

                # elr[l, n] = sum_hc alr[hc, l] * featT[hc, n]
                pe = pse.tile([2 * H, CHUNK], F32, tag="pe")
                nc.tensor.matmul(pe[:], alr_b[:], fTb[:], start=True, stop=True)
                eT = wp.tile([2 * H, CHUNK], F16, tag="eT")
                nc.vector.tensor_copy(eT[:], pe[:])

                nc.sync.dma_start(featT_d[:, cs], fT[:])
                nc.sync.dma_start(elrT_d[:, cs], eT[:])
    nc.compile()
    nc.m = get_hw_module(nc.m)
    return nc


# --------------------------------------------------------------------------
# Launch 2: edge aggregation.  meta = dict with block structure.
# --------------------------------------------------------------------------
def build_launch2(meta):
    nb = meta["nb"]                # [TILES] blocks per tile (common per core)
    NBS_MAX = meta["nbs_max"]      # max blocks in one supertile
    BTOT = meta["btot"]            # total blocks
    sup_base = meta["sup_base"]    # block col base per supertile
    PT = HC + 8                    # psum cols per tile (132 used, pad to 136)

    nc = bacc.Bacc("TRN2", target_bir_lowering=False, debug=False,
                   num_devices=NCORES)
    rhs_d = nc.dram_tensor("rhs", [128, BTOT * HC], F16, kind="ExternalInput")
    elr_d = nc.dram_tensor("elr", [128, BTOT * 2 * H], F16, kind="ExternalInput")
    mt_d = nc.dram_tensor("mt8", [128, BTOT * TW], FP8, kind="ExternalInput")
    bias_d = nc.dram_tensor("bias", [1, HC], F32, kind="ExternalInput")
    ones_d = nc.dram_tensor("ones", [1, TW], F32, kind="ExternalInput")
    out_d = nc.dram_tensor("out", [NPAD2, C], F32, kind="ExternalOutput")

    with tile.TileContext(nc) as tc:
        with (
            tc.tile_pool(name="const", bufs=1) as cp,
            tc.tile_pool(name="sup", bufs=3) as sp,
            tc.tile_pool(name="bk", bufs=3) as bp,
            tc.tile_pool(name="col", bufs=2) as colp,
            tc.tile_pool(name="ep", bufs=2) as ep,
            tc.tile_pool(name="pso", bufs=4, space="PSUM") as pso,
            tc.tile_pool(name="psb", bufs=1, space="PSUM") as psb,
        ):
            # bias: mean over heads, broadcast to TW partitions
            bsb = cp.tile([1, HC], F32)
            nc.sync.dma_start(bsb[:], bias_d[:])
            ones = cp.tile([1, TW], F32)
            nc.sync.dma_start(ones[:], ones_d[:])
            b01 = cp.tile([1, C], F32)
            nc.vector.tensor_add(b01[:], bsb[:, 0:C], bsb[:, C:2 * C])
            b23 = cp.tile([1, C], F32)
            nc.vector.tensor_add(b23[:], bsb[:, 2 * C:3 * C], bsb[:, 3 * C:4 * C])
            bsum = cp.tile([1, C], F32)
            nc.vector.tensor_add(bsum[:], b01[:], b23[:])
            bmean = cp.tile([1, C], F32)
            nc.vector.tensor_scalar_mul(bmean[:], bsum[:], 0.25)
            pb = psb.tile([TW, C], F32)
            nc.tensor.matmul(pb[:], ones[:], bmean[:], start=True, stop=True)
            biasb = cp.tile([TW, C], F32)
            nc.vector.tensor_copy(biasb[:], pb[:])

            for sb in range(NSUP // SBATCH):
                # collect numerators/denominators for SBATCH supertiles
                coll = colp.tile([TW, SBATCH * STILE, PT], F32, tag="coll")
                ntile = 0           # tiles collected in this batch
                t0 = (sb * SBATCH * STILE)      # first global tile
                for si in range(SBATCH):
                    s = sb * SBATCH + si
                    ts = list(range(s * STILE, min((s + 1) * STILE, TILES)))
                    nbs = sum(nb[t] for t in ts)
                    cb = sup_base[s]

                    rhs_sb = sp.tile([128, NBS_MAX, HC], F16, tag="rhs")
                    nc.sync.dma_start(
                        rhs_sb[:, 0:nbs, :].rearrange("p b c -> p (b c)"),
                        rhs_d[:, cb * HC:(cb + nbs) * HC])
                    elr_sb = sp.tile([128, NBS_MAX, 2 * H], F16, tag="elr")
                    nc.sync.dma_start(
                        elr_sb[:, 0:nbs, :].rearrange("p b h -> p (b h)"),
                        elr_d[:, cb * 2 * H:(cb + nbs) * 2 * H])
                    mt_sb = sp.tile([128, NBS_MAX, TW], FP8, tag="mt")
                    nc.sync.dma_start(
                        mt_sb[:, 0:nbs, :].rearrange("p b d -> p (b d)"),
                        mt_d[:, cb * TW:(cb + nbs) * TW])

                    # e2 = leakyrelu(el + er)  [128, nbs, H] f32
                    e_sb = bp.tile([128, NBS_MAX * H], F32, tag="e")
                    nc.vector.tensor_tensor(
                        out=e_sb[:, 0:nbs * H].rearrange(
                            "p (b h) -> p b h", h=H),
                        in0=elr_sb[:, 0:nbs, 0:H],
                        in1=elr_sb[:, 0:nbs, H:2 * H],
                        op=mybir.AluOpType.add)
                    t1 = bp.tile([128, NBS_MAX * H], F32, tag="t1")
                    nc.vector.tensor_scalar_mul(
                        t1[:, 0:nbs * H], e_sb[:, 0:nbs * H], NEG)
                    e2 = bp.tile([128, NBS_MAX * H], F32, tag="e2")
                    nc.vector.tensor_tensor(
                        out=e2[:, 0:nbs * H], in0=e_sb[:, 0:nbs * H],
                        in1=t1[:, 0:nbs * H], op=mybir.AluOpType.max)
                    ex = bp.tile([128, NBS_MAX * H], F16, tag="ex")
                    nc.scalar.activation(ex[:, 0:nbs * H], e2[:, 0:nbs * H],
                                         mybir.ActivationFunctionType.Exp)

                    # rhs2 = [feat * ex | ex]  (fp16; feat is c-major)
                    rhs2 = bp.tile([128, NBS_MAX, HC + H], F16, tag="rhs2")
                    ex_bc = bass.AP(
                        ex.tensor, ex[:].offset,
                        [ex[:].ap[0], [H, nbs], [0, C], [1, H]])
                    nc.vector.tensor_tensor(
                        out=rhs2[:, 0:nbs, 0:HC].rearrange(
                            "p b (c h) -> p b c h", h=H),
                        in0=rhs_sb[:, 0:nbs, :].rearrange(
                            "p b (c h) -> p b c h", h=H),
                        in1=ex_bc, op=mybir.AluOpType.mult)
                    nc.vector.tensor_copy(
                        rhs2[:, 0:nbs, HC:HC + H],
                        ex[:, 0:nbs * H].rearrange("p (b h) -> p b h", h=H))

                    # aggregate into per-supertile psum [TW, STILE, PT]
                    pout = pso.tile([TW, STILE, PT], F32, tag="pout",
                                    name=f"pout{s}")
                    off = 0
                    for tl, t in enumerate(ts):
                        cnt = nb[t]
                        for j in range(cnt):
                            jb = off + j
                            nc.tensor.matmul(
                                pout[:, tl, 0:HC + H], mt_sb[:, jb, :],
                                rhs2[:, jb, :],
                                start=(j == 0), stop=(j == cnt - 1),
                                skip_group_check=True)
                        off += cnt
                    # move to collect buffer (scalar engine; frees psum)
                    nc.scalar.copy(
                        coll[:, si * STILE:si * STILE + len(ts), :],
                        pout[:, 0:len(ts), :])
                    ntile += len(ts)

                # ---- batched epilogue over ntile tiles ----
                nt = ntile
                s4 = ep.tile([TW, SBATCH * STILE, H], F32, tag="s4")
                nc.vector.tensor_scalar(
                    out=s4[:, 0:nt, :], in0=coll[:, 0:nt, HC:HC + H],
                    scalar1=4.0, scalar2=1e-20,
                    op0=mybir.AluOpType.mult, op1=mybir.AluOpType.add)
                srec = ep.tile([TW, SBATCH * STILE, H], F32, tag="srec")
                nc.vector.reciprocal_approx_fast(
                    srec[:, 0:nt, :], s4[:, 0:nt, :])
                scaled = ep.tile([TW, SBATCH * STILE, C, H], F32, tag="scaled")
                srec_bc = bass.AP(
                    srec.tensor, srec[:].offset,
                    [srec[:].ap[0], [H, nt], [0, C], [1, H]])
                nc.vector.tensor_tensor(
                    out=scaled[:, 0:nt, :, :],
                    in0=coll[:, 0:nt, 0:HC].rearrange(
                        "p b (c h) -> p b c h", h=H),
                    in1=srec_bc, op=mybir.AluOpType.mult)
                # mean over heads (c-major: heads are innermost)
                h2 = ep.tile([TW, SBATCH * STILE, C, 2], F32, tag="h2")
                nc.vector.tensor_tensor(
                    out=h2[:, 0:nt, :, :],
                    in0=scaled[:, 0:nt, :, 0:2],
                    in1=scaled[:, 0:nt, :, 2:4],
                    op=mybir.AluOpType.add)
                hs = ep.tile([TW, SBATCH * STILE, C], F32, tag="hs")
                nc.vector.tensor_tensor(
                    out=hs[:, 0:nt, :], in0=h2[:, 0:nt, :, 0],
                    in1=h2[:, 0:nt, :, 1], op=mybir.AluOpType.add)
                hb = ep.tile([TW, SBATCH * STILE, C], F32, tag="hb")
                biasb_bc = bass.AP(
                    biasb.tensor, biasb[:].offset,
                    [biasb[:].ap[0], [0, nt], [1, C]])
                nc.vector.tensor_tensor(
                    out=hb[:, 0:nt, :], in0=hs[:, 0:nt, :], in1=biasb_bc,
                    op=mybir.AluOpType.add)
                outt = ep.tile([TW, SBATCH * STILE, C], F32, tag="outt")
                nc.scalar.activation(
                    outt[:, 0:nt, :].rearrange("p b c -> p (b c)"),
                    hb[:, 0:nt, :].rearrange("p b c -> p (b c)"),
                    mybir.ActivationFunctionType.Relu)
                nc.sync.dma_start(
                    out_d[t0 * TW:(t0 + nt) * TW, :].rearrange(
                        "(b p) c -> p b c", p=TW),
                    outt[:, 0:nt, :])
    nc.compile()
    nc.m = get_hw_module(nc.m)
    return nc


# --------------------------------------------------------------------------
# Host-side routing (pure integer work)
# --------------------------------------------------------------------------
def balance_tiles(owner, dloc):
    """Assign each core's 12,500 nodes to 196 64-wide dst tiles, balancing
    per-tile in-edge counts; align tile labels across cores by load rank.
    Returns perm[NCORES, NPC]: local node -> tile*64 + slot."""
    perm = np.zeros((NCORES, NPC), np.int64)
    target = 1024
    for k in range(NCORES):
        m = owner == k
        deg = np.bincount(dloc[m], minlength=NPC)
        order = np.argsort(-deg, kind="stable")
        load = np.zeros(TILES, np.int64)
        cnt = np.zeros(TILES, np.int64)
        cap = np.full(TILES, TW, np.int64)
        cap[TILES - 1] = NPC - (TILES - 1) * TW
        assign = np.zeros(NPC, np.int64)
        for n in order:
            d = deg[n]
            avail = np.where(cnt < cap)[0]
            fits = avail[load[avail] + d <= target]
            pool = fits if len(fits) else avail
            t = pool[np.argmin(load[pool])]
            load[t] += d
            assign[n] = t
            cnt[t] += 1
        # relabel tiles by descending load so ranks align across cores
        rank_of = np.empty(TILES, np.int64)
        rank_of[np.argsort(-load, kind="stable")] = np.arange(TILES)
        # keep the short last tile's label (different capacity)
        lastr = rank_of[TILES - 1]
        swap_t = np.where(rank_of == TILES - 1)[0][0]
        rank_of[TILES - 1], rank_of[swap_t] = TILES - 1, lastr
        assign = rank_of[assign]
        slots = np.zeros(TILES, np.int64)
        for n in range(NPC):
            t = assign[n]
            perm[k, n] = t * TW + slots[t]
            slots[t] += 1
    return perm


def route_edges(src, dst):
    """Bucket edges by (owner core, dst tile); pad to common 128-edge
    blocks.  Returns meta + per-core slot arrays."""
    src = src.astype(np.int64)
    dst = dst.astype(np.int64)
    owner = dst // NPC
    dloc = dst - owner * NPC

    perm = balance_tiles(owner, dloc)
    slot = perm[owner, dloc]            # balanced slot of each edge's dst
    t_id = slot // TW
    doff = slot - t_id * TW

    cnt = np.bincount(owner * TILES + t_id,
                      minlength=NCORES * TILES).reshape(NCORES, TILES)
    nb = np.ceil(cnt.max(axis=0) / 128.0).astype(np.int64)    # [TILES]
    btot = int(nb.sum())
    epad = btot * 128

    boff = np.zeros(TILES, np.int64)
    boff[1:] = np.cumsum(nb)[:-1]
    sup_base = []
    nbs_max = 0
    for s in range(NSUP):
        ts = list(range(s * STILE, min((s + 1) * STILE, TILES)))
        sup_base.append(int(boff[ts[0]]))
        nbs_max = max(nbs_max, int(sum(nb[t] for t in ts)))

    key = owner * TILES + t_id
    order = np.argsort(key, kind="stable")
    key_s = key[order]
    doff_s = doff[order]
    src_s = src[order]
    dst_s = dst[order]

    srow_all = np.full((NCORES, epad), N, np.int64)   # N -> zero pad row
    drow_all = np.full((NCORES, epad), N, np.int64)
    doff_all = np.full((NCORES, epad), -1, np.int64)

    core_starts = np.searchsorted(key_s // TILES, np.arange(NCORES + 1))
    for k in range(NCORES):
        a, b = core_starts[k], core_starts[k + 1]
        kk = key_s[a:b] % TILES                       # tile ids, sorted
        starts = np.searchsorted(kk, np.arange(TILES))
        rank = np.arange(b - a) - starts[kk]
        pos = boff[kk] * 128 + rank
        srow_all[k, pos] = src_s[a:b]
        drow_all[k, pos] = dst_s[a:b]
        doff_all[k, pos] = doff_s[a:b]

    meta = {
        "nb": nb.tolist(),
        "nbs_max": int(nbs_max),
        "btot": btot,
        "sup_base": sup_base,
    }
    return meta, srow_all, drow_all, doff_all, perm


# --------------------------------------------------------------------------
def kernel(x, src, dst, W, attn_l, attn_r, bias):
    x = np.asarray(x, dtype=np.float32)
    src = np.asarray(src)
    dst = np.asarray(dst)
    W = np.asarray(W, dtype=np.float32)
    attn_l = np.asarray(attn_l, dtype=np.float32)
    attn_r = np.asarray(attn_r, dtype=np.float32)
    bias = np.asarray(bias, dtype=np.float32)

    meta, srow_all, drow_all, doff_all, perm = route_edges(src, dst)

    # ---- launch 1 ----
    if "l1" not in _cache:
        _cache["l1"] = build_launch1()
    nc1 = _cache["l1"]

    # c-major column permutation of W (host-side column reorder)
    cmperm = np.array([(j % H) * C + j // H for j in range(HC)], np.int64)
    W_cm = np.ascontiguousarray(W[:, cmperm])
    xt = np.ascontiguousarray(x.T)                     # [256, 100000]
    alr = np.zeros((HC, 2 * H), np.float32)            # block-diag attn layout
    for h in range(H):
        alr[h * C:(h + 1) * C, h] = attn_l[h]
        alr[h * C:(h + 1) * C, H + h] = attn_r[h]
    alr_cm = np.ascontiguousarray(alr[cmperm, :])

    in1 = []
    for k in range(NCORES):
        xtk = np.zeros((IN, NPAD1), np.float32)
        xtk[:, :NPC] = xt[:, k * NPC:(k + 1) * NPC]
        in1.append({"xt": xtk, "w": W_cm, "alr": alr_cm})
    res1 = run_bass_kernel_spmd(nc1, in1, list(range(NCORES)))

    # feat rows [N+1, HC] fp16 c-major (+ zero pad row); el/er rows fp16
    feat_rows = np.zeros((N + 1, HC), np.float16)
    el_rows = np.zeros((N + 1, H), np.float16)
    er_rows = np.zeros((N + 1, H), np.float16)
    for k in range(NCORES):
        cs = slice(k * NPC, (k + 1) * NPC)
        feat_rows[cs] = res1.results[k]["featT"][:, :NPC].T
        elr = res1.results[k]["elrT"][:, :NPC]
        el_rows[cs] = elr[0:H].T
        er_rows[cs] = elr[H:2 * H].T

    # ---- launch 2 inputs (host: pure indexing / byte gathers) ----
    key2 = (meta["btot"], meta["nbs_max"], tuple(meta["nb"]))
    if ("l2", key2) not in _cache:
        _cache[("l2", key2)] = build_launch2(meta)
    nc2 = _cache[("l2", key2)]

    btot = meta["btot"]
    bias2 = bias.reshape(1, HC)
    ones = np.ones((1, TW), np.float32)

    in2 = []
    for k in range(NCORES):
        srow_T = np.ascontiguousarray(
            srow_all[k].reshape(btot, 128).T)          # [128, btot]
        drow_T = np.ascontiguousarray(drow_all[k].reshape(btot, 128).T)
        rhs = feat_rows.take(srow_T, axis=0)           # [128, btot, 128] fp16
        elr = np.concatenate(
            [el_rows.take(srow_T, axis=0),
             er_rows.take(drow_T, axis=0)], axis=2)    # [128, btot, 8] fp16
        # one-hot dst matrices as fp8 {0,1}
        mt8 = np.zeros((128, btot, TW), ml_dtypes.float8_e4m3)
        doff_T = doff_all[k].reshape(btot, 128).T      # [128, btot]
        pp, bb = np.nonzero(doff_T >= 0)
        mt8[pp, bb, doff_T[pp, bb]] = 1.0
        in2.append({
            "rhs": np.ascontiguousarray(rhs.reshape(128, btot * HC)),
            "elr": np.ascontiguousarray(elr.reshape(128, btot * 2 * H)),
            "mt8": mt8.reshape(128, btot * TW),
            "bias": bias2,
            "ones": ones,
        })
    res2 = run_bass_kernel_spmd(nc2, in2, list(range(NCORES)))

    out = np.concatenate(
        [res2.results[k]["out"][perm[k]] for k in range(NCORES)])
    return out.astype(np.float32)
